# revision 13
# baseline (speedup 1.0000x reference)
"""CapsuleLayer forward (squash + per-capsule matmul) on 8 Trainium2 cores.

Reference computation (all fp32):
    x  = inputs.reshape(B, 1152, 8)
    pc = squash(x)                              # per-(b,n) over k=8
    u_hat[b,n,j,d] = sum_k W[0,n,j,d,k] * pc[b,n,k]
    out = u_hat[..., None]                      # [B, 1152, 10, 16, 1]

Sharding: capsule dim (n=1152) split 144-per-core across 8 cores; every core
keeps the full batch (B=512).  Zero cross-device communication.

Per-core kernel (v3 — DMA-roofline oriented):
  - weights host-packed as 4-cap block-diagonal [32, 640] fp16 sub-blocks.
    On device each 16-cap group's [128, 2560] block-diagonal tile is
    zero-filled once (DVE/Pool, fp16 2x) and the four sub-blocks land in
    their diagonal positions by strided sub-block DMA — no engine copies.
  - squash on Pool(x^2) + ACT(sqrt) + DVE(reduce, reciprocal_approx_fast
    x2, muls)
  - pc transposed to [ck, b] via PE transpose, 4 transposes packed per
    PSUM bank, one quad CAST each to SBUF fp16
  - fp16 K=128 matmuls (1 cycle/row) into [128, 1536] 3-bank PSUM units
  - PSUM->SBUF evacuation split per unit: ACT takes cols [0,SPLIT),
    DVE takes [SPLIT,1536)
  - fp16 output, 6 KiB-row HWDGE stores (two units per store) on sync
  - chunk pipeline: next chunk's squash+transposes emitted mid-chunk so
    PE flows across chunk boundaries without stalling
"""

from contextlib import ExitStack

import numpy as np

import concourse.bacc as bacc
import concourse.bass as bass  # noqa: F401  (AP helpers)
import concourse.mybir as mybir
import concourse.tile as tile
from concourse.bass_utils import run_bass_kernel_spmd
from concourse.masks import make_identity

N_CORES = 8
B = 512
N_CAPS = 1152
K = 8
JD = 160  # 10*16
CAPS_PER_CORE = N_CAPS // N_CORES  # 144
GROUP_CAPS = 16  # caps per pcT transpose group -> K=128 rows
N_GROUPS = CAPS_PER_CORE // GROUP_CAPS  # 9
GROUP_COLS = GROUP_CAPS * JD  # 2560
P = 128
B_CHUNKS = B // P  # 4
EPS = 1e-07
SUB_CAPS = 4  # caps per 32-partition diag sub-block
SUB_P = SUB_CAPS * K  # 32
SUB_COLS = SUB_CAPS * JD  # 640
N_SUBS = GROUP_CAPS // SUB_CAPS  # 4
CHUNK_COLS = CAPS_PER_CORE * JD  # 23040
UNIT = 1536  # matmul/evac unit: 3 PSUM banks
N_UNITS = CHUNK_COLS // UNIT  # 15
SPLIT = 960  # ACT evacuates [0,SPLIT), DVE [SPLIT,UNIT)
STORE_COLS = 2 * UNIT  # 3072

F32 = mybir.dt.float32
F16 = mybir.dt.float16
OUT_DT = mybir.dt.float16
OUT_NP = np.float16

# Matmul windows per unit: one K=128 matmul per 512-col PSUM bank against
# the block-diagonal [128, 2560] fp16 weight tile of the owning group.
# (g, dst_lo, rhs_lo): dst relative to the unit, rhs relative to group g.
UNIT_PIECES = [[] for _ in range(N_UNITS)]
for _w in range(CHUNK_COLS // 512):
    _col = _w * 512
    _g = _col // GROUP_COLS
    _u = _col // UNIT
    UNIT_PIECES[_u].append((_g, _col - _u * UNIT, _col - _g * GROUP_COLS))


def build_program():
    nc = bacc.Bacc("TRN2", debug=False, num_devices=N_CORES)
    x = nc.dram_tensor("x", [B, CAPS_PER_CORE * K], F32, kind="ExternalInput").ap()
    wt = nc.dram_tensor(
        "wt", [CAPS_PER_CORE * K, SUB_COLS], F16, kind="ExternalInput"
    ).ap()
    out = nc.dram_tensor(
        "out", [B, CAPS_PER_CORE * JD], OUT_DT, kind="ExternalOutput"
    ).ap()

    with tile.TileContext(nc) as tc, ExitStack() as ctx:
        consts = ctx.enter_context(tc.tile_pool(name="consts", bufs=1))
        wblk_pool = ctx.enter_context(tc.tile_pool(name="wblk", bufs=1))
        xpool = ctx.enter_context(tc.tile_pool(name="xpool", bufs=2))
        x2pool = ctx.enter_context(tc.tile_pool(name="x2pool", bufs=2))
        stats = ctx.enter_context(tc.tile_pool(name="stats", bufs=2))
        pcpool = ctx.enter_context(tc.tile_pool(name="pcpool", bufs=2))
        pct_pool = ctx.enter_context(tc.tile_pool(name="pct", bufs=6))
        ost_pool = ctx.enter_context(tc.tile_pool(name="ost", bufs=6))
        tp_psum = ctx.enter_context(tc.tile_pool(name="tp_psum", bufs=2, space="PSUM"))
        pm_psum = ctx.enter_context(tc.tile_pool(name="pm_psum", bufs=2, space="PSUM"))

        identity = consts.tile([P, P], F32)
        make_identity(nc, identity)
        eps_tile = consts.tile([P, 1], F32)
        nc.vector.memset(eps_tile, EPS)
        zero_col = consts.tile([P, 1], F32)
        nc.vector.memset(zero_col, 0.0)

        # One resident fp16 block-diagonal weight tile [128, 23040]: bulk
        # zero-fill (DVE for the head, Pool for the tail), then 4 strided
        # DMAs per group-range land every [32, 640] sub-block directly in
        # its diagonal position.
        wblk_all = wblk_pool.tile([P, N_GROUPS * N_SUBS * SUB_COLS], F16)
        wt_g = wt.rearrange("(g q r) c -> q r g c", g=N_GROUPS, q=N_SUBS)

        def build_wblk(g0, g1):
            # zero-fill cols [g0*2560, g1*2560) then DMA subs for g0..g1-1
            ng = g1 - g0
            fill = wblk_all[:, g0 * GROUP_COLS : g1 * GROUP_COLS]
            if g0 == 0:
                nc.vector.tensor_copy(
                    out=fill, in_=zero_col.broadcast_to([P, ng * GROUP_COLS])
                )
            else:
                nc.gpsimd.memset(fill, 0.0)
            for q in range(N_SUBS):
                dst = wblk_all[
                    q * SUB_P : (q + 1) * SUB_P,
                    g0 * GROUP_COLS : g1 * GROUP_COLS,
                ].rearrange("p (g c) -> p g c", c=GROUP_COLS)[
                    :, :, q * SUB_COLS : (q + 1) * SUB_COLS
                ]
                nc.scalar.dma_start(out=dst, in_=wt_g[q, :, g0:g1, :])

        def load_x(bi):
            xt = xpool.tile([P, CAPS_PER_CORE, K], F32)
            nc.scalar.dma_start(
                out=xt,
                in_=x[bi * P : (bi + 1) * P, :].rearrange("b (c k) -> b c k", k=K),
            )
            return xt

        def squash(xt):
            # scale[b,c] = (sq/(1+sq)) / sqrt(sq+eps);  pc = x*scale
            x2 = x2pool.tile([P, CAPS_PER_CORE, K], F32)
            nc.gpsimd.tensor_mul(x2, xt, xt)
            sq = stats.tile([P, CAPS_PER_CORE], F32)
            nc.vector.reduce_sum(out=sq, in_=x2, axis=mybir.AxisListType.X)
            sn = stats.tile([P, CAPS_PER_CORE], F32)
            nc.scalar.activation(
                out=sn, in_=sq, func=mybir.ActivationFunctionType.Sqrt,
                bias=eps_tile, scale=1.0,
            )
            rn = stats.tile([P, CAPS_PER_CORE], F32)
            nc.vector.reciprocal_approx_fast(rn, sn)
            t1 = stats.tile([P, CAPS_PER_CORE], F32)
            nc.vector.tensor_scalar_add(t1, sq, 1.0)
            r2 = stats.tile([P, CAPS_PER_CORE], F32)
            nc.vector.reciprocal_approx_fast(r2, t1)
            m1 = stats.tile([P, CAPS_PER_CORE], F32)
            nc.vector.tensor_mul(m1, sq, rn)
            scale = stats.tile([P, CAPS_PER_CORE], F32)
            nc.vector.tensor_mul(scale, m1, r2)
            pc = pcpool.tile([P, CAPS_PER_CORE, K], F32)
            nc.vector.tensor_mul(
                pc, xt, scale.unsqueeze(2).broadcast_to([P, CAPS_PER_CORE, K])
            )
            return pc

        def transposes(pc):
            # 9 PE transposes -> 3 PSUM quad banks -> 3 fp16 SBUF quad tiles
            pc_flat = pc.rearrange("p c k -> p (c k)")
            quads = []
            for qi in range(3):
                n = 4 if qi < 2 else 1
                tp = tp_psum.tile([P, 512], F32)
                for j in range(n):
                    g = qi * 4 + j
                    nc.tensor.transpose(
                        tp[:, j * P : (j + 1) * P],
                        pc_flat[:, g * P : (g + 1) * P],
                        identity,
                    )
                pct = pct_pool.tile([P, 512], F16)
                nc.vector.tensor_copy(pct[:, : n * P], tp[:, : n * P])
                quads.append(pct)
            return quads

        def do_unit(bi, u, quads, cur_ost):
            if u % 2 == 0:
                cur_ost = ost_pool.tile([P, STORE_COLS], OUT_DT)
            pm = pm_psum.tile([P, UNIT], F32)
            for (g, dlo, rlo) in UNIT_PIECES[u]:
                pct = quads[g // 4]
                sl = (g % 4) * P
                nc.tensor.matmul(
                    pm[:, dlo : dlo + 512],
                    lhsT=pct[:, sl : sl + P],
                    rhs=wblk_all[:, g * GROUP_COLS + rlo : g * GROUP_COLS + rlo + 512],
                    start=True,
                    stop=True,
                )
            off = (u % 2) * UNIT
            nc.scalar.copy(cur_ost[:, off : off + SPLIT], pm[:, :SPLIT])
            nc.vector.tensor_copy(cur_ost[:, off + SPLIT : off + UNIT], pm[:, SPLIT:])
            if u % 2 == 1 or u == N_UNITS - 1:
                w = STORE_COLS if u % 2 == 1 else UNIT
                c0 = (u // 2) * STORE_COLS
                nc.sync.dma_start(
                    out=out[bi * P : (bi + 1) * P, c0 : c0 + w],
                    in_=cur_ost[:, :w],
                )
            return cur_ost

        xt = [None] * B_CHUNKS
        quads = [None] * B_CHUNKS
        xt[0] = load_x(0)
        if B_CHUNKS > 1:
            xt[1] = load_x(1)
        build_wblk(0, 2)
        build_wblk(2, N_GROUPS)
        quads[0] = transposes(squash(xt[0]))
        for bi in range(B_CHUNKS):
            cur_ost = None
            for u in range(N_UNITS):
                cur_ost = do_unit(bi, u, quads[bi], cur_ost)
                if u == 7 and bi + 1 < B_CHUNKS:
                    if bi + 2 < B_CHUNKS:
                        xt[bi + 2] = load_x(bi + 2)
                    quads[bi + 1] = transposes(squash(xt[bi + 1]))
    nc.compile()
    return nc


_PROGRAM = None


def _get_program():
    global _PROGRAM
    if _PROGRAM is None:
        _PROGRAM = build_program()
    return _PROGRAM


def shard_inputs(inputs: np.ndarray, W: np.ndarray) -> list[dict[str, np.ndarray]]:
    # W -> k-major [n, k, jd], packed as 4-cap diagonal fp16 sub-blocks:
    # wtb[(g,q,ci,k), ci*JD+jd] = W[0][n, jd, k]; zeros off-diagonal.
    wt_kmaj = np.asarray(W[0], dtype=np.float32).reshape(N_CAPS, JD, K)
    wt_kmaj = wt_kmaj.transpose(0, 2, 1)  # [n, k, jd]
    n_sub_total = N_CAPS // SUB_CAPS
    sub = wt_kmaj.reshape(n_sub_total, SUB_CAPS, K, JD)
    wtb = np.zeros((n_sub_total, SUB_CAPS, K, SUB_COLS), dtype=np.float16)
    for ci in range(SUB_CAPS):
        wtb[:, ci, :, ci * JD : (ci + 1) * JD] = sub[:, ci].astype(np.float16)
    wtb = wtb.reshape(N_CAPS * K, SUB_COLS)
    in_maps = []
    for i in range(N_CORES):
        c0 = i * CAPS_PER_CORE
        in_maps.append(
            {
                "x": np.ascontiguousarray(
                    inputs[:, c0 * K : (c0 + CAPS_PER_CORE) * K], dtype=np.float32
                ),
                "wt": np.ascontiguousarray(
                    wtb[c0 * K : (c0 + CAPS_PER_CORE) * K]
                ),
            }
        )
    return in_maps


def unshard_output(results: list[dict[str, np.ndarray]]) -> np.ndarray:
    full = np.empty((B, N_CAPS, JD), dtype=np.float32)
    for i in range(N_CORES):
        c0 = i * CAPS_PER_CORE
        full[:, c0 : c0 + CAPS_PER_CORE, :] = results[i]["out"].reshape(
            B, CAPS_PER_CORE, JD
        ).astype(np.float32)
    return full.reshape(B, N_CAPS, 10, 16, 1)


def kernel(inputs: np.ndarray, W: np.ndarray) -> np.ndarray:
    nc = _get_program()
    in_maps = shard_inputs(np.asarray(inputs), np.asarray(W))
    res = run_bass_kernel_spmd(nc, in_maps, core_ids=list(range(N_CORES)))
    return unshard_output(res.results)


# revision 14
# speedup vs baseline: 1.0529x; 1.0529x over previous
"""CapsuleLayer forward (squash + per-capsule matmul) on 8 Trainium2 cores.

Reference computation (all fp32):
    x  = inputs.reshape(B, 1152, 8)
    pc = squash(x)                              # per-(b,n) over k=8
    u_hat[b,n,j,d] = sum_k W[0,n,j,d,k] * pc[b,n,k]
    out = u_hat[..., None]                      # [B, 1152, 10, 16, 1]

Sharding: capsule dim (n=1152) split 144-per-core across 8 cores; every core
keeps the full batch (B=512).  Zero cross-device communication.

Per-core kernel (v3 — DMA-roofline oriented):
  - weights host-packed as 4-cap block-diagonal [32, 640] fp16 sub-blocks.
    On device each 16-cap group's [128, 2560] block-diagonal tile is
    zero-filled once (DVE/Pool, fp16 2x) and the four sub-blocks land in
    their diagonal positions by strided sub-block DMA — no engine copies.
  - squash on Pool(x^2) + ACT(sqrt) + DVE(reduce, reciprocal_approx_fast
    x2, muls)
  - pc transposed to [ck, b] via PE transpose, 4 transposes packed per
    PSUM bank, one quad CAST each to SBUF fp16
  - fp16 K=128 matmuls (1 cycle/row) into [128, 1536] 3-bank PSUM units
  - PSUM->SBUF evacuation split per unit: ACT takes cols [0,SPLIT),
    DVE takes [SPLIT,1536)
  - fp16 output, 6 KiB-row HWDGE stores (two units per store) on sync
  - chunk pipeline: next chunk's squash+transposes emitted mid-chunk so
    PE flows across chunk boundaries without stalling
"""

from contextlib import ExitStack

import numpy as np

import concourse.bacc as bacc
import concourse.bass as bass  # noqa: F401  (AP helpers)
import concourse.mybir as mybir
import concourse.tile as tile
from concourse.bass_utils import run_bass_kernel_spmd
from concourse.masks import make_identity

N_CORES = 8
B = 512
N_CAPS = 1152
K = 8
JD = 160  # 10*16
CAPS_PER_CORE = N_CAPS // N_CORES  # 144
GROUP_CAPS = 16  # caps per pcT transpose group -> K=128 rows
N_GROUPS = CAPS_PER_CORE // GROUP_CAPS  # 9
GROUP_COLS = GROUP_CAPS * JD  # 2560
P = 128
B_CHUNKS = B // P  # 4
EPS = 1e-07
SUB_CAPS = 4  # caps per 32-partition diag sub-block
SUB_P = SUB_CAPS * K  # 32
SUB_COLS = SUB_CAPS * JD  # 640
N_SUBS = GROUP_CAPS // SUB_CAPS  # 4
CHUNK_COLS = CAPS_PER_CORE * JD  # 23040
UNIT = 1536  # matmul/evac unit: 3 PSUM banks
N_UNITS = CHUNK_COLS // UNIT  # 15
SPLIT = 1024  # ACT evacuates [0,SPLIT), DVE [SPLIT,UNIT)
STORE_COLS = 2 * UNIT  # 3072

F32 = mybir.dt.float32
F16 = mybir.dt.float16
OUT_DT = mybir.dt.float16
OUT_NP = np.float16

# Matmul windows per unit: one K=128 matmul per 512-col PSUM bank against
# the block-diagonal [128, 2560] fp16 weight tile of the owning group.
# (g, dst_lo, rhs_lo): dst relative to the unit, rhs relative to group g.
UNIT_PIECES = [[] for _ in range(N_UNITS)]
for _w in range(CHUNK_COLS // 512):
    _col = _w * 512
    _g = _col // GROUP_COLS
    _u = _col // UNIT
    UNIT_PIECES[_u].append((_g, _col - _u * UNIT, _col - _g * GROUP_COLS))


def build_program():
    nc = bacc.Bacc("TRN2", debug=False, num_devices=N_CORES)
    x = nc.dram_tensor("x", [B, CAPS_PER_CORE * K], F32, kind="ExternalInput").ap()
    wt = nc.dram_tensor(
        "wt", [CAPS_PER_CORE * K, SUB_COLS], F16, kind="ExternalInput"
    ).ap()
    out = nc.dram_tensor(
        "out", [B, CAPS_PER_CORE * JD], OUT_DT, kind="ExternalOutput"
    ).ap()

    with tile.TileContext(nc) as tc, ExitStack() as ctx:
        consts = ctx.enter_context(tc.tile_pool(name="consts", bufs=1))
        wblk_pool = ctx.enter_context(tc.tile_pool(name="wblk", bufs=1))
        xpool = ctx.enter_context(tc.tile_pool(name="xpool", bufs=2))
        x2pool = ctx.enter_context(tc.tile_pool(name="x2pool", bufs=2))
        stats = ctx.enter_context(tc.tile_pool(name="stats", bufs=2))
        pcpool = ctx.enter_context(tc.tile_pool(name="pcpool", bufs=2))
        pct_pool = ctx.enter_context(tc.tile_pool(name="pct", bufs=6))
        ost_pool = ctx.enter_context(tc.tile_pool(name="ost", bufs=6))
        tp_psum = ctx.enter_context(tc.tile_pool(name="tp_psum", bufs=2, space="PSUM"))
        pm_psum = ctx.enter_context(tc.tile_pool(name="pm_psum", bufs=2, space="PSUM"))

        identity = consts.tile([P, P], F32)
        make_identity(nc, identity)
        eps_tile = consts.tile([P, 1], F32)
        nc.vector.memset(eps_tile, EPS)
        zero_col = consts.tile([P, 1], F32)
        nc.vector.memset(zero_col, 0.0)

        # One resident fp16 block-diagonal weight tile [128, 23040]: bulk
        # zero-fill (DVE for the head, Pool for the tail), then 4 strided
        # DMAs per group-range land every [32, 640] sub-block directly in
        # its diagonal position.
        wblk_all = wblk_pool.tile([P, N_GROUPS * N_SUBS * SUB_COLS], F16)
        wt_g = wt.rearrange("(g q r) c -> q r g c", g=N_GROUPS, q=N_SUBS)

        built = [False] * N_GROUPS

        def build_wblk(g):
            # zero-fill group g's [128, 2560] range, then 4 sub-block DMAs
            fill = wblk_all[:, g * GROUP_COLS : (g + 1) * GROUP_COLS]
            if g % 2 == 0:
                nc.vector.tensor_copy(
                    out=fill, in_=zero_col.broadcast_to([P, GROUP_COLS])
                )
            else:
                nc.gpsimd.memset(fill, 0.0)
            for q in range(N_SUBS):
                dst = wblk_all[
                    q * SUB_P : (q + 1) * SUB_P,
                    g * GROUP_COLS : (g + 1) * GROUP_COLS,
                ][:, q * SUB_COLS : (q + 1) * SUB_COLS]
                nc.sync.dma_start(out=dst, in_=wt_g[q, :, g, :])
            built[g] = True

        def load_x(bi):
            xt = xpool.tile([P, CAPS_PER_CORE, K], F32)
            nc.sync.dma_start(
                out=xt,
                in_=x[bi * P : (bi + 1) * P, :].rearrange("b (c k) -> b c k", k=K),
            )
            return xt

        def squash(xt):
            # scale[b,c] = (sq/(1+sq)) / sqrt(sq+eps);  pc = x*scale
            x2 = x2pool.tile([P, CAPS_PER_CORE, K], F32)
            nc.gpsimd.tensor_mul(x2, xt, xt)
            sq = stats.tile([P, CAPS_PER_CORE], F32)
            nc.vector.reduce_sum(out=sq, in_=x2, axis=mybir.AxisListType.X)
            sn = stats.tile([P, CAPS_PER_CORE], F32)
            nc.scalar.activation(
                out=sn, in_=sq, func=mybir.ActivationFunctionType.Sqrt,
                bias=eps_tile, scale=1.0,
            )
            rn = stats.tile([P, CAPS_PER_CORE], F32)
            nc.vector.reciprocal_approx_fast(rn, sn)
            t1 = stats.tile([P, CAPS_PER_CORE], F32)
            nc.vector.tensor_scalar_add(t1, sq, 1.0)
            r2 = stats.tile([P, CAPS_PER_CORE], F32)
            nc.vector.reciprocal_approx_fast(r2, t1)
            m1 = stats.tile([P, CAPS_PER_CORE], F32)
            nc.vector.tensor_mul(m1, sq, rn)
            scale = stats.tile([P, CAPS_PER_CORE], F32)
            nc.vector.tensor_mul(scale, m1, r2)
            pc = pcpool.tile([P, CAPS_PER_CORE, K], F32)
            nc.vector.tensor_mul(
                pc, xt, scale.unsqueeze(2).broadcast_to([P, CAPS_PER_CORE, K])
            )
            return pc

        def transposes(pc):
            # 9 PE transposes -> 3 PSUM quad banks -> 3 fp16 SBUF quad tiles
            pc_flat = pc.rearrange("p c k -> p (c k)")
            quads = []
            for qi in range(3):
                n = 4 if qi < 2 else 1
                tp = tp_psum.tile([P, 512], F32)
                for j in range(n):
                    g = qi * 4 + j
                    nc.tensor.transpose(
                        tp[:, j * P : (j + 1) * P],
                        pc_flat[:, g * P : (g + 1) * P],
                        identity,
                    )
                pct = pct_pool.tile([P, 512], F16)
                nc.vector.tensor_copy(pct[:, : n * P], tp[:, : n * P])
                quads.append(pct)
            return quads

        def do_unit(bi, u, quads, cur_ost):
            if u % 2 == 0:
                cur_ost = ost_pool.tile([P, STORE_COLS], OUT_DT)
            pm = pm_psum.tile([P, UNIT], F32)
            for (g, dlo, rlo) in UNIT_PIECES[u]:
                pct = quads[g // 4]
                sl = (g % 4) * P
                nc.tensor.matmul(
                    pm[:, dlo : dlo + 512],
                    lhsT=pct[:, sl : sl + P],
                    rhs=wblk_all[:, g * GROUP_COLS + rlo : g * GROUP_COLS + rlo + 512],
                    start=True,
                    stop=True,
                )
            off = (u % 2) * UNIT
            nc.scalar.copy(cur_ost[:, off : off + SPLIT], pm[:, :SPLIT])
            nc.vector.tensor_copy(cur_ost[:, off + SPLIT : off + UNIT], pm[:, SPLIT:])
            if u % 2 == 1 or u == N_UNITS - 1:
                w = STORE_COLS if u % 2 == 1 else UNIT
                c0 = (u // 2) * STORE_COLS
                nc.sync.dma_start(
                    out=out[bi * P : (bi + 1) * P, c0 : c0 + w],
                    in_=cur_ost[:, :w],
                )
            return cur_ost

        xt = [None] * B_CHUNKS
        quads = [None] * B_CHUNKS
        xt[0] = load_x(0)
        if B_CHUNKS > 1:
            xt[1] = load_x(1)
        build_wblk(0)
        build_wblk(1)
        quads[0] = transposes(squash(xt[0]))
        for bi in range(B_CHUNKS):
            cur_ost = None
            for u in range(N_UNITS):
                ga = min((u * UNIT + 3 * UNIT) // GROUP_COLS, N_GROUPS - 1)
                if bi == 0 and not built[ga]:
                    build_wblk(ga)
                cur_ost = do_unit(bi, u, quads[bi], cur_ost)
                if u == 7 and bi + 1 < B_CHUNKS:
                    if bi + 2 < B_CHUNKS:
                        xt[bi + 2] = load_x(bi + 2)
                    quads[bi + 1] = transposes(squash(xt[bi + 1]))
    nc.compile()
    return nc


_PROGRAM = None


def _get_program():
    global _PROGRAM
    if _PROGRAM is None:
        _PROGRAM = build_program()
    return _PROGRAM


def shard_inputs(inputs: np.ndarray, W: np.ndarray) -> list[dict[str, np.ndarray]]:
    # W -> k-major [n, k, jd], packed as 4-cap diagonal fp16 sub-blocks:
    # wtb[(g,q,ci,k), ci*JD+jd] = W[0][n, jd, k]; zeros off-diagonal.
    wt_kmaj = np.asarray(W[0], dtype=np.float32).reshape(N_CAPS, JD, K)
    wt_kmaj = wt_kmaj.transpose(0, 2, 1)  # [n, k, jd]
    n_sub_total = N_CAPS // SUB_CAPS
    sub = wt_kmaj.reshape(n_sub_total, SUB_CAPS, K, JD)
    wtb = np.zeros((n_sub_total, SUB_CAPS, K, SUB_COLS), dtype=np.float16)
    for ci in range(SUB_CAPS):
        wtb[:, ci, :, ci * JD : (ci + 1) * JD] = sub[:, ci].astype(np.float16)
    wtb = wtb.reshape(N_CAPS * K, SUB_COLS)
    in_maps = []
    for i in range(N_CORES):
        c0 = i * CAPS_PER_CORE
        in_maps.append(
            {
                "x": np.ascontiguousarray(
                    inputs[:, c0 * K : (c0 + CAPS_PER_CORE) * K], dtype=np.float32
                ),
                "wt": np.ascontiguousarray(
                    wtb[c0 * K : (c0 + CAPS_PER_CORE) * K]
                ),
            }
        )
    return in_maps


def unshard_output(results: list[dict[str, np.ndarray]]) -> np.ndarray:
    full = np.empty((B, N_CAPS, JD), dtype=np.float32)
    for i in range(N_CORES):
        c0 = i * CAPS_PER_CORE
        full[:, c0 : c0 + CAPS_PER_CORE, :] = results[i]["out"].reshape(
            B, CAPS_PER_CORE, JD
        ).astype(np.float32)
    return full.reshape(B, N_CAPS, 10, 16, 1)


def kernel(inputs: np.ndarray, W: np.ndarray) -> np.ndarray:
    nc = _get_program()
    in_maps = shard_inputs(np.asarray(inputs), np.asarray(W))
    res = run_bass_kernel_spmd(nc, in_maps, core_ids=list(range(N_CORES)))
    return unshard_output(res.results)
